# revision 49
# baseline (speedup 1.0000x reference)
"""Causal self-attention (B=2, T=2048, D=768, H=12) on 8 TRN2 cores.

Sharding: core r handles batch b=r%2 and head-group g=r//2 (3 heads).
Replica groups are parity-matched ({0,2,4,6} batch 0, {1,3,5,7} batch 1):
observed core-boot skew is odd/even structured, so each 4-core group
syncs only among cores that boot together, and the auto start-barrier
(AllGather over merged replica groups) never couples the two groups.

Per query block bi (512 tokens), fully pipelined:
  1. qkv projection for the block (this core's 3 heads).  x is loaded
     from a host-blocked [4*768, 512] layout so every DMA is contiguous.
  2. attention: S^T = K.Q with keys on partitions, heads 0/1 sharing one
     [128,1024] PSUM strip (one exp ACT covers both), head 2 pairing
     consecutive j-tiles.  The AV matmul for j-tile tj-1 is emitted
     after the S matmuls for tj so the in-order PE never waits on the
     scalar-engine exp.  exp(S/8) without max-subtraction; denominator
     via a ones-row appended to V (O^T = V_aug^T E).
  3. normalize O^T out of PSUM (vector reciprocal on the [1,512]
     denominator row, gpsimd partition-broadcast, vector multiply);
     qkv(bi+1) is emitted before proj(bi) so the PE chews on it while
     the normalize chain runs on the other engines.
  4. partial proj over this core's 192 features -> y_part [512,768] f16,
     DMA'd straight to DRAM.  No collectives at all: the host-side
     assemble() sums the four head-group partials per batch in fp32
     (the unshard step), so cores never synchronize and per-core boot
     skew cannot inflate the max-core exec time.
"""

import numpy as np

import concourse.bass as bass
import concourse.bacc as bacc
import concourse.mybir as mybir
import concourse.tile as tile
from concourse.bass_utils import run_bass_kernel_spmd

F32 = mybir.dt.float32
F16 = mybir.dt.float16

B, T, D = 2, 2048, 768
H, DH = 12, 64
NCORES = 8
HPC = H // 4          # heads per core = 3
QK = HPC * DH         # 192 rows of q (or k) per core
KC = D // 128         # 6 contraction chunks
NBI = T // 512        # 4 query blocks

EXP_SCALE = 1.0 / np.sqrt(DH)  # 0.125
# v_aug column strides.  Heads 0/1: 64 v dims + one ones column (M=65 AV
# matmuls are cheaper; their normalize chain overlaps the head-2 section).
# Head 2: 64 v dims + 64 replicated ones columns, so its AV lands the
# denominator broadcast across PSUM rows 64:128 and the (exposed, block-
# boundary) normalize collapses to copy+reciprocal+multiply with no
# gpsimd partition_broadcast in the chain.
VW01 = 65
VW2 = 128


def _emit(tc, aps):
    nc = tc.nc
    xT, wqkT, wvT, wpT, triu, y = (
        aps["xT"], aps["wqkT"], aps["wvT"], aps["wpT"], aps["triu"], aps["y"])

    pools = []

    def pool(name, bufs, space="SBUF"):
        p = tc.tile_pool(name=name, bufs=bufs, space=space)
        pools.append(p)
        return p.__enter__()

    consts = pool("consts", 1)
    xw = pool("xw", 1)
    qk_sb = pool("qk_sb", 1)
    v_sb = pool("v_sb", 1)
    work = pool("work", 3)
    norm = pool("norm", 2)
    ot_sb = pool("ot_sb", 2)
    ysb = pool("ysb", 4)
    ps = pool("ps", 2, space="PSUM")
    ps_o = pool("ps_o", 1, space="PSUM")
    ps_v = pool("ps_v", 1, space="PSUM")

    # ---- input loads: everything host-swizzled to partition-major so each
    # tensor arrives in ONE dma_start (descriptor streams stay contiguous and
    # the per-call issue cost on the engine queues collapses).
    # x layout: [128, NBI*KC*512]; col (bi*KC+k)*512+t = x[b][bi*512+t, k*128+p]
    x_sb = xw.tile([128, NBI * KC * 512], F16, tag="x", name="x")
    wqk_sb = consts.tile([128, KC * 2 * QK], F16, tag="wqk", name="wqk")
    wv_sb = consts.tile([128, KC * QK], F16, tag="wv", name="wv")

    def xcol(bi, k):  # column base of (block, k-chunk) in x_sb
        return (bi * KC + k) * 512

    # critical path first: wqk k-chunk 0 + per-chunk x(b0) calls alternating
    # sync/scalar, so chunk k lands k-th and the k-outer first-block qkv
    # consumes each on arrival; x blocks 1-3 are issued from the BACK of the
    # gpsimd queue (after the boot memsets) so their 2.4 MB never competes
    # with block-0 latency.
    nc.gpsimd.dma_start(wqk_sb[:, 0:2 * QK], wqkT[:, 0:2 * QK])
    nc.sync.dma_start(x_sb[:, 0:512], xT[:, 0:512])
    nc.scalar.dma_start(x_sb[:, 512:1024], xT[:, 512:1024])
    nc.gpsimd.dma_start(wqk_sb[:, 2 * QK:], wqkT[:, 2 * QK:])
    for k in range(2, KC):
        (nc.sync if k % 2 == 0 else nc.scalar).dma_start(
            x_sb[:, k * 512:(k + 1) * 512], xT[:, k * 512:(k + 1) * 512])
    triu_sb = consts.tile([128, 128], F16, tag="triu", name="triu")
    nc.gpsimd.dma_start(wv_sb[:], wvT[:, :])
    nc.gpsimd.dma_start(triu_sb[:], triu[:, :])
    wpA_sb = consts.tile([128, D], F16, tag="wpA", name="wpA")
    wpB_sb = consts.tile([64, D], F16, tag="wpB", name="wpB")

    # ---- persistent q/k/v tiles ----
    # heads 0/1 packed into [128, T] (rows 0-63 / 64-127); head 2 in [64, T].
    qTp = qk_sb.tile([128, T], F16, tag="qTp", name="qTp")
    kTp = qk_sb.tile([128, T], F16, tag="kTp", name="kTp")
    # head 2 q/k live in BOTH partition halves: paired j-tiles then hit
    # disjoint PE row groups (h0/h64) and run concurrently, like heads 0/1.
    qT2 = qk_sb.tile([128, T], F16, tag="qT2", name="qT2")
    kT2 = qk_sb.tile([128, T], F16, tag="kT2", name="kT2")
    VWS = [VW01, VW01, VW2]
    v_aug = [v_sb.tile([128, (T // 128) * VWS[h]], F16, tag=f"v{h}", name=f"v{h}")
             for h in range(HPC)]
    # static ones columns, set once at boot; only the v-data halves are
    # rewritten per block.  Emitted on the gpsimd queue ahead of the
    # deferred x DMAs below.
    for h in range(HPC):
        vw = VWS[h]
        for tt in range(T // 128):
            nc.gpsimd.memset(v_aug[h][:, tt * vw + 64:(tt + 1) * vw], 1.0)
    # deferred bulk loads ride the back of the gpsimd queue: x blocks 1-3
    # with the proj weights (first needed ~25us in) slotted between
    nc.gpsimd.dma_start(
        x_sb[:, xcol(1, 0):xcol(2, 0)], xT[:, xcol(1, 0):xcol(2, 0)])
    nc.gpsimd.dma_start(wpA_sb[:], wpT[0:128, :])
    nc.gpsimd.dma_start(wpB_sb[:], wpT[128:QK, :])
    for bi in range(2, NBI):
        nc.gpsimd.dma_start(
            x_sb[:, xcol(bi, 0):xcol(bi + 1, 0)], xT[:, xcol(bi, 0):xcol(bi + 1, 0)])

    def _qk_mm(bi, m):
        p = ps.tile([128, 1024], F32, tag="s", name="qkps")[:, 0:512]
        for k in range(KC):
            nc.tensor.matmul(
                p[:],
                wqk_sb[:, k * 2 * QK + m * 128:k * 2 * QK + (m + 1) * 128],
                x_sb[:, xcol(bi, k):xcol(bi, k) + 512],
                start=(k == 0), stop=(k == KC - 1))
        return p

    def emit_qkv_m0(bi):
        # m0's cast rides scalar, so emitting it before the last AV2 flush
        # leaves the vector queue free for the triu mask that gates AV2.
        ns = slice(bi * 512, (bi + 1) * 512)
        p = _qk_mm(bi, 0)
        nc.scalar.copy(qTp[:, ns], p[:])

    def emit_qkv_m12(bi):
        ns = slice(bi * 512, (bi + 1) * 512)
        p = _qk_mm(bi, 1)
        nc.vector.tensor_copy(qT2[0:64, ns], p[0:64, :])
        nc.vector.tensor_copy(qT2[64:128, ns], p[0:64, :])
        nc.vector.tensor_copy(kTp[0:64, ns], p[64:128, :])
        p = _qk_mm(bi, 2)
        nc.scalar.copy(kTp[64:128, ns], p[0:64, :])
        nc.scalar.copy(kT2[0:64, ns], p[64:128, :])
        nc.scalar.copy(kT2[64:128, ns], p[64:128, :])

    def emit_qkv_v(bi):
        for tt in range(bi * 4, bi * 4 + 4):
            # own single-bank pool, two region slots: never contends with the
            # m-strip casts for PSUM write-after-read
            p = ps_v.tile([128, 512], F32, tag="v", name="vps")[
                :, (tt % 2) * 256:(tt % 2) * 256 + QK]
            u = tt % 4
            for k in range(KC):
                nc.tensor.matmul(
                    p[:],
                    x_sb[:, xcol(bi, k) + u * 128:xcol(bi, k) + (u + 1) * 128],
                    wv_sb[:, k * QK:(k + 1) * QK],
                    start=(k == 0), stop=(k == KC - 1))
            for h in range(HPC):
                nc.vector.tensor_copy(
                    v_aug[h][:, tt * VWS[h]:tt * VWS[h] + 64],
                    p[:, h * 64:(h + 1) * 64])

    def normalize01(o_ps, dst, i):
        # heads 0/1: single denominator row.  Stays entirely off the scalar
        # queue (exp must not sit behind these) and overlaps the head-2
        # section / next-block qkv on the PE.
        den = norm.tile([1, 512], F32, tag=f"den{i}", name=f"den{i}")
        nc.vector.tensor_copy(den[:], o_ps[64:65, :])
        rec = norm.tile([1, 512], F32, tag=f"rec{i}", name=f"rec{i}")
        # den > 0 always (the exp terms are positive), so approx_fast is safe
        nc.vector.reciprocal_approx_fast(rec[:], den[:])
        rb = norm.tile([64, 512], F32, tag=f"rb{i}", name=f"rb{i}")
        nc.gpsimd.partition_broadcast(rb[:], rec[:])
        nc.vector.tensor_mul(dst, o_ps[0:64, :], rb[:])

    def normalize2(o_ps, dst):
        # head 2: denominator already replicated across PSUM rows 64:128.
        # The copy rides scalar (free right after the last exp) so only
        # reciprocal+multiply occupy the vector queue.
        den = norm.tile([64, 512], F32, tag="den2", name="den2")
        nc.scalar.copy(den[:], o_ps[64:128, :])
        rec = norm.tile([64, 512], F32, tag="rec2", name="rec2")
        nc.vector.reciprocal_approx_fast(rec[:], den[:])
        nc.vector.tensor_mul(dst, o_ps[0:64, :], rec[:])

    emit_qkv_m0(0)
    emit_qkv_m12(0)
    emit_qkv_v(0)
    for bi in range(NBI):
        ntj = 4 * bi + 4
        o01 = [ps_o.tile([65, 512], F32, tag=f"o{h}", name=f"o{h}") for h in range(2)]
        o2 = ps_o.tile([128, 512], F32, tag="o2", name="o2")
        OT01 = ot_sb.tile([128, 512], F16, tag="OT01", name="OT01")
        OT2 = ot_sb.tile([64, 512], F16, tag="OT2", name="OT2")

        # ---- heads 0/1: S(tj) then AV(tj-1), one exp per j-tile ----
        pend = None  # (e_tile, tj, lo)

        def flush_av():
            e, tj, lo = pend
            for h in range(2):
                if tj - 4 * bi >= 0:
                    nc.vector.tensor_mul(
                        e[:, h * 512 + lo:h * 512 + lo + 128],
                        e[:, h * 512 + lo:h * 512 + lo + 128], triu_sb[:])
                nc.tensor.matmul(
                    o01[h][:, lo:],
                    v_aug[h][:, tj * VW01:(tj + 1) * VW01],
                    e[:, h * 512 + lo:(h + 1) * 512],
                    start=(tj == 0), stop=(tj == ntj - 1))

        for tj in range(ntj):
            dtile = tj - 4 * bi
            lo = max(dtile, 0) * 128
            js = slice(tj * 128, (tj + 1) * 128)
            qs = slice(bi * 512 + lo, (bi + 1) * 512)
            s_ps = ps.tile([128, 1024], F32, tag="s", name="s")
            nc.tensor.matmul(s_ps[:, lo:512], kTp[0:64, js], qTp[0:64, qs],
                             start=True, stop=True)
            nc.tensor.matmul(s_ps[:, 512 + lo:1024], kTp[64:128, js], qTp[64:128, qs],
                             start=True, stop=True)
            e = work.tile([128, 1024], F16, tag="e", name="e")
            if lo == 0:
                nc.scalar.activation(e[:], s_ps[:],
                                     mybir.ActivationFunctionType.Exp, scale=EXP_SCALE)
            else:
                for h in range(2):
                    nc.scalar.activation(
                        e[:, h * 512 + lo:(h + 1) * 512],
                        s_ps[:, h * 512 + lo:(h + 1) * 512],
                        mybir.ActivationFunctionType.Exp, scale=EXP_SCALE)
            if pend is not None:
                flush_av()
            pend = (e, tj, lo)
        flush_av()

        # ---- head 2: paired j-tiles, AV one pair behind ----
        def flush_av2(ep, pp, losp):
            for idx, tj in enumerate(pp):
                if tj - 4 * bi >= 0:
                    nc.vector.tensor_mul(
                        ep[:, idx * 512 + losp[idx]:idx * 512 + losp[idx] + 128],
                        ep[:, idx * 512 + losp[idx]:idx * 512 + losp[idx] + 128],
                        triu_sb[:])
                nc.tensor.matmul(
                    o2[:, losp[idx]:],
                    v_aug[2][:, tj * VW2:(tj + 1) * VW2],
                    ep[:, idx * 512 + losp[idx]:(idx + 1) * 512],
                    start=(tj == 0), stop=(tj == ntj - 1))

        pend2 = None  # (e, pair, los)
        for tj0 in range(0, ntj, 2):
            pair = (tj0, tj0 + 1)
            s_ps = ps.tile([128, 1024], F32, tag="s", name="s2")
            e = work.tile([128, 1024], F16, tag="e", name="e2")
            los = []
            for idx, tj in enumerate(pair):
                lo = max(tj - 4 * bi, 0) * 128
                los.append(lo)
                hs = slice(64 * idx, 64 * idx + 64)
                js = slice(tj * 128, (tj + 1) * 128)
                qs = slice(bi * 512 + lo, (bi + 1) * 512)
                nc.tensor.matmul(
                    s_ps[:, idx * 512 + lo:(idx + 1) * 512],
                    kT2[hs, js], qT2[hs, qs],
                    start=True, stop=True)
            if los[1] == 0:
                nc.scalar.activation(e[:], s_ps[:],
                                     mybir.ActivationFunctionType.Exp, scale=EXP_SCALE)
            else:
                for idx in range(2):
                    nc.scalar.activation(
                        e[:, idx * 512 + los[idx]:(idx + 1) * 512],
                        s_ps[:, idx * 512 + los[idx]:(idx + 1) * 512],
                        mybir.ActivationFunctionType.Exp, scale=EXP_SCALE)
            if pend2 is not None:
                flush_av2(*pend2)
            pend2 = (e, pair, los)
        # drain: the next block's qkv m0 is emitted BEFORE the final AV2
        # flush so the PE chews on it while the last exp runs (m0's cast is
        # scalar-side, so the flush's vector triu mask is not queued behind
        # it); m1/m2 and the v-loop follow, then the normalize chains.
        if bi + 1 < NBI:
            emit_qkv_m0(bi + 1)
        flush_av2(*pend2)
        if bi + 1 < NBI:
            emit_qkv_m12(bi + 1)
        # normalize chains ahead of the v-loop casts on the vector queue:
        # proj (gated by the muls) is due sooner than v_aug (next block's
        # diagonal AVs, ~10us away).
        normalize01(o01[0], OT01[0:64], 0)
        normalize01(o01[1], OT01[64:128], 1)
        normalize2(o2, OT2[:, :])
        if bi + 1 < NBI:
            emit_qkv_v(bi + 1)

        # ---- partial proj, straight to DRAM (host sums the partials) ----
        # pj tiles come from the big rotating ps pool, so consecutive tt's
        # overlap (matmuls on one buffer while the casts drain the other).
        # Casts split across vector+scalar; output DMAs ride the idle sync
        # queue.
        for tt in range(4):
            ts = slice(tt * 128, (tt + 1) * 128)
            y_t = ysb.tile([128, D], F16, tag="y_t", name="y_t")
            pj = ps.tile([128, 1024], F32, tag="s", name="pj")
            # matmul outputs may not cross a PSUM bank: 512 cols in bank 0,
            # the remaining 256 in bank 1 of the same rotating tile.
            for on, osz in ((0, 512), (512, 256)):
                nc.tensor.matmul(
                    pj[:, on:on + osz], OT01[:, ts], wpA_sb[:, on:on + osz],
                    start=True, stop=False)
                nc.tensor.matmul(
                    pj[:, on:on + osz], OT2[:, ts], wpB_sb[:, on:on + osz],
                    start=False, stop=True)
            nc.vector.tensor_copy(y_t[:, 0:384], pj[:, 0:384])
            nc.scalar.copy(y_t[:, 384:D], pj[:, 384:D])
            # each half leaves as soon as its own cast lands
            yrow = slice(bi * 512 + tt * 128, bi * 512 + (tt + 1) * 128)
            nc.sync.dma_start(y[yrow, 0:384], y_t[:, 0:384])
            nc.gpsimd.dma_start(y[yrow, 384:D], y_t[:, 384:D])

    for p in reversed(pools):
        p.__exit__(None, None, None)


_NC_CACHE = {}


def _get_nc():
    if "nc" in _NC_CACHE:
        return _NC_CACHE["nc"]
    nc = bacc.Bacc("TRN2", num_devices=NCORES, debug=False)
    aps = {
        "xT": nc.dram_tensor(
            "xT", [128, NBI * KC * 512], F16, kind="ExternalInput").ap(),
        "wqkT": nc.dram_tensor(
            "wqkT", [128, KC * 2 * QK], F16, kind="ExternalInput").ap(),
        "wvT": nc.dram_tensor(
            "wvT", [128, KC * QK], F16, kind="ExternalInput").ap(),
        "wpT": nc.dram_tensor("wpT", [QK, D], F16, kind="ExternalInput").ap(),
        "triu": nc.dram_tensor("triu", [128, 128], F16, kind="ExternalInput").ap(),
        "y": nc.dram_tensor("y", [T, D], F16, kind="ExternalOutput").ap(),
    }
    with tile.TileContext(nc, num_cores=NCORES) as tc:
        _emit(tc, aps)
    nc.compile()
    _NC_CACHE["nc"] = nc
    return nc


def make_in_maps(x, W_qkv, W_proj):
    triu = np.triu(np.ones((128, 128), dtype=np.float16))
    wpT_full = np.ascontiguousarray(W_proj.T).astype(np.float16)  # [in, out]
    in_maps = []
    for r in range(NCORES):
        b, g = r % 2, r // 2
        rs = slice(QK * g, QK * (g + 1))
        wq = W_qkv[0:D][rs]
        wk = W_qkv[D:2 * D][rs]
        wv = W_qkv[2 * D:3 * D][rs]
        # partition-major folds: row p holds k-chunk blocks back to back, so
        # each tensor arrives in a single contiguous-descriptor dma_start.
        wqkT = np.ascontiguousarray(
            np.concatenate([wq, wk], axis=0).T.astype(np.float16)
            .reshape(KC, 128, 2 * QK).transpose(1, 0, 2).reshape(128, KC * 2 * QK))
        wvT = np.ascontiguousarray(
            wv.T.astype(np.float16)
            .reshape(KC, 128, QK).transpose(1, 0, 2).reshape(128, KC * QK))
        wpT = np.ascontiguousarray(wpT_full[rs, :])
        # x: [128, NBI*KC*512]; col (bi*KC+k)*512+t = x[b][bi*512+t, k*128+p]
        xT = np.ascontiguousarray(
            x[b].astype(np.float16).reshape(NBI, 512, KC, 128)
            .transpose(3, 0, 2, 1).reshape(128, NBI * KC * 512))
        in_maps.append({"xT": xT, "wqkT": wqkT, "wvT": wvT,
                        "wpT": wpT, "triu": triu})
    return in_maps


def assemble(results):
    # unshard: per batch, sum the four head-group partial projections
    y = np.zeros((B, T, D), dtype=np.float32)
    for r in range(NCORES):
        b = r % 2
        y[b] += results[r]["y"].astype(np.float32)
    return y


def kernel(**inputs):
    x = np.asarray(inputs["x"], dtype=np.float32)
    W_qkv = np.asarray(inputs["W_qkv"], dtype=np.float32)
    W_proj = np.asarray(inputs["W_proj"], dtype=np.float32)
    nc = _get_nc()
    in_maps = make_in_maps(x, W_qkv, W_proj)
    res = run_bass_kernel_spmd(nc, in_maps, core_ids=list(range(NCORES)))
    return assemble(res.results)



# revision 53
# speedup vs baseline: 1.0432x; 1.0432x over previous
"""Causal self-attention (B=2, T=2048, D=768, H=12) on 8 TRN2 cores.

Sharding: core r handles batch b=r%2 and head-group g=r//2 (3 heads).
Replica groups are parity-matched ({0,2,4,6} batch 0, {1,3,5,7} batch 1):
observed core-boot skew is odd/even structured, so each 4-core group
syncs only among cores that boot together, and the auto start-barrier
(AllGather over merged replica groups) never couples the two groups.

Per query block bi (512 tokens), fully pipelined:
  1. qkv projection for the block (this core's 3 heads).  x is loaded
     from a host-blocked [4*768, 512] layout so every DMA is contiguous.
  2. attention: S^T = K.Q with keys on partitions, heads 0/1 sharing one
     [128,1024] PSUM strip (one exp ACT covers both), head 2 pairing
     consecutive j-tiles.  The AV matmul for j-tile tj-1 is emitted
     after the S matmuls for tj so the in-order PE never waits on the
     scalar-engine exp.  exp(S/8) without max-subtraction; denominator
     via a ones-row appended to V (O^T = V_aug^T E).
  3. normalize O^T out of PSUM (vector reciprocal on the [1,512]
     denominator row, gpsimd partition-broadcast, vector multiply);
     qkv(bi+1) is emitted before proj(bi) so the PE chews on it while
     the normalize chain runs on the other engines.
  4. partial proj over this core's 192 features -> y_part [512,768] f16,
     DMA'd straight to DRAM.  No collectives at all: the host-side
     assemble() sums the four head-group partials per batch in fp32
     (the unshard step), so cores never synchronize and per-core boot
     skew cannot inflate the max-core exec time.
"""

import numpy as np

import concourse.bass as bass
import concourse.bacc as bacc
import concourse.mybir as mybir
import concourse.tile as tile
from concourse.bass_utils import run_bass_kernel_spmd

F32 = mybir.dt.float32
F16 = mybir.dt.float16

B, T, D = 2, 2048, 768
H, DH = 12, 64
NCORES = 8
HPC = H // 4          # heads per core = 3
QK = HPC * DH         # 192 rows of q (or k) per core
KC = D // 128         # 6 contraction chunks
NBI = T // 512        # 4 query blocks

EXP_SCALE = 1.0 / np.sqrt(DH)  # 0.125
# v_aug column strides.  Heads 0/1: 64 v dims + one ones column (M=65 AV
# matmuls are cheaper; their normalize chain overlaps the head-2 section).
# Head 2: 64 v dims + 64 replicated ones columns, so its AV lands the
# denominator broadcast across PSUM rows 64:128 and the (exposed, block-
# boundary) normalize collapses to copy+reciprocal+multiply with no
# gpsimd partition_broadcast in the chain.
VW01 = 65
VW2 = 128


def _emit(tc, aps):
    nc = tc.nc
    xT, wqkT, wvT, wpT, triu, y = (
        aps["xT"], aps["wqkT"], aps["wvT"], aps["wpT"], aps["triu"], aps["y"])

    pools = []

    def pool(name, bufs, space="SBUF"):
        p = tc.tile_pool(name=name, bufs=bufs, space=space)
        pools.append(p)
        return p.__enter__()

    consts = pool("consts", 1)
    xw = pool("xw", 1)
    qk_sb = pool("qk_sb", 1)
    v_sb = pool("v_sb", 1)
    work = pool("work", 3)
    norm = pool("norm", 2)
    ot_sb = pool("ot_sb", 2)
    ysb = pool("ysb", 4)
    ps = pool("ps", 2, space="PSUM")
    ps_o = pool("ps_o", 1, space="PSUM")
    ps_v = pool("ps_v", 1, space="PSUM")

    # ---- input loads: everything host-swizzled to partition-major so each
    # tensor arrives in ONE dma_start (descriptor streams stay contiguous and
    # the per-call issue cost on the engine queues collapses).
    # x layout: [128, NBI*KC*512]; col (bi*KC+k)*512+t = x[b][bi*512+t, k*128+p]
    x_sb = xw.tile([128, NBI * KC * 512], F16, tag="x", name="x")
    wqk_sb = consts.tile([128, KC * 2 * QK], F16, tag="wqk", name="wqk")
    wv_sb = consts.tile([128, KC * QK], F16, tag="wv", name="wv")

    def xcol(bi, k):  # column base of (block, k-chunk) in x_sb
        return (bi * KC + k) * 512

    # critical path first: wqk k-chunk 0 + per-chunk x(b0) calls alternating
    # sync/scalar, so chunk k lands k-th and the k-outer first-block qkv
    # consumes each on arrival; x blocks 1-3 are issued from the BACK of the
    # gpsimd queue (after the boot memsets) so their 2.4 MB never competes
    # with block-0 latency.
    nc.gpsimd.dma_start(wqk_sb[:, 0:2 * QK], wqkT[:, 0:2 * QK])
    nc.sync.dma_start(x_sb[:, 0:512], xT[:, 0:512])
    nc.scalar.dma_start(x_sb[:, 512:1024], xT[:, 512:1024])
    nc.gpsimd.dma_start(wqk_sb[:, 2 * QK:], wqkT[:, 2 * QK:])
    for k in range(2, KC):
        (nc.sync if k % 2 == 0 else nc.scalar).dma_start(
            x_sb[:, k * 512:(k + 1) * 512], xT[:, k * 512:(k + 1) * 512])
    triu_sb = consts.tile([128, 128], F16, tag="triu", name="triu")
    nc.gpsimd.dma_start(wv_sb[:], wvT[:, :])
    nc.gpsimd.dma_start(triu_sb[:], triu[:, :])
    wpA_sb = consts.tile([128, D], F16, tag="wpA", name="wpA")
    wpB_sb = consts.tile([64, D], F16, tag="wpB", name="wpB")

    # ---- persistent q/k/v tiles ----
    # heads 0/1 packed into [128, T] (rows 0-63 / 64-127); head 2 in [64, T].
    qTp = qk_sb.tile([128, T], F16, tag="qTp", name="qTp")
    kTp = qk_sb.tile([128, T], F16, tag="kTp", name="kTp")
    # head 2 q/k live in BOTH partition halves: paired j-tiles then hit
    # disjoint PE row groups (h0/h64) and run concurrently, like heads 0/1.
    qT2 = qk_sb.tile([128, T], F16, tag="qT2", name="qT2")
    kT2 = qk_sb.tile([128, T], F16, tag="kT2", name="kT2")
    VWS = [VW01, VW01, VW2]
    v_aug = [v_sb.tile([128, (T // 128) * VWS[h]], F16, tag=f"v{h}", name=f"v{h}")
             for h in range(HPC)]
    # static ones columns, set once at boot; only the v-data halves are
    # rewritten per block.  Emitted on the gpsimd queue ahead of the
    # deferred x DMAs below.
    for h in range(HPC):
        vw = VWS[h]
        for tt in range(T // 128):
            nc.gpsimd.memset(v_aug[h][:, tt * vw + 64:(tt + 1) * vw], 1.0)
    # deferred bulk loads ride the back of the gpsimd queue: x blocks 1-3
    # with the proj weights (first needed ~25us in) slotted between
    nc.gpsimd.dma_start(
        x_sb[:, xcol(1, 0):xcol(2, 0)], xT[:, xcol(1, 0):xcol(2, 0)])
    nc.gpsimd.dma_start(wpA_sb[:], wpT[0:128, :])
    nc.gpsimd.dma_start(wpB_sb[:], wpT[128:QK, :])
    for bi in range(2, NBI):
        nc.gpsimd.dma_start(
            x_sb[:, xcol(bi, 0):xcol(bi + 1, 0)], xT[:, xcol(bi, 0):xcol(bi + 1, 0)])

    def _qk_mm(bi, m):
        p = ps.tile([128, 1024], F32, tag="s", name="qkps")[:, 0:512]
        for k in range(KC):
            nc.tensor.matmul(
                p[:],
                wqk_sb[:, k * 2 * QK + m * 128:k * 2 * QK + (m + 1) * 128],
                x_sb[:, xcol(bi, k):xcol(bi, k) + 512],
                start=(k == 0), stop=(k == KC - 1))
        return p

    def emit_qkv_m0(bi):
        # m0's cast rides scalar, so emitting it before the last AV2 flush
        # leaves the vector queue free for the triu mask that gates AV2.
        ns = slice(bi * 512, (bi + 1) * 512)
        p = _qk_mm(bi, 0)
        nc.scalar.copy(qTp[:, ns], p[:])

    def emit_qkv_m1_mm(bi):
        # m1 accumulates in the ps_v bank: unlike the rotating ps pool, its
        # WAR is on the long-done v casts, so these matmuls can run during
        # the S2 exp drain when emitted before the final AV2 flush.
        p = ps_v.tile([128, 512], F32, tag="v", name="m1ps")
        for k in range(KC):
            nc.tensor.matmul(
                p[:],
                wqk_sb[:, k * 2 * QK + 128:k * 2 * QK + 256],
                x_sb[:, xcol(bi, k):xcol(bi, k) + 512],
                start=(k == 0), stop=(k == KC - 1))
        return p

    def emit_qkv_m12_rest(bi, p1):
        ns = slice(bi * 512, (bi + 1) * 512)
        nc.vector.tensor_copy(qT2[0:64, ns], p1[0:64, :])
        nc.vector.tensor_copy(qT2[64:128, ns], p1[0:64, :])
        nc.vector.tensor_copy(kTp[0:64, ns], p1[64:128, :])
        p = _qk_mm(bi, 2)
        nc.scalar.copy(kTp[64:128, ns], p[0:64, :])
        nc.scalar.copy(kT2[0:64, ns], p[64:128, :])
        nc.scalar.copy(kT2[64:128, ns], p[64:128, :])

    def emit_qkv_v(bi):
        for tt in range(bi * 4, bi * 4 + 4):
            # own single-bank pool, two region slots: never contends with the
            # m-strip casts for PSUM write-after-read
            p = ps_v.tile([128, 512], F32, tag="v", name="vps")[
                :, (tt % 2) * 256:(tt % 2) * 256 + QK]
            u = tt % 4
            for k in range(KC):
                nc.tensor.matmul(
                    p[:],
                    x_sb[:, xcol(bi, k) + u * 128:xcol(bi, k) + (u + 1) * 128],
                    wv_sb[:, k * QK:(k + 1) * QK],
                    start=(k == 0), stop=(k == KC - 1))
            for h in range(HPC):
                nc.vector.tensor_copy(
                    v_aug[h][:, tt * VWS[h]:tt * VWS[h] + 64],
                    p[:, h * 64:(h + 1) * 64])

    def normalize01(o_ps, dst, i):
        # heads 0/1: single denominator row.  Stays entirely off the scalar
        # queue (exp must not sit behind these) and overlaps the head-2
        # section / next-block qkv on the PE.
        den = norm.tile([1, 512], F32, tag=f"den{i}", name=f"den{i}")
        nc.vector.tensor_copy(den[:], o_ps[64:65, :])
        rec = norm.tile([1, 512], F32, tag=f"rec{i}", name=f"rec{i}")
        # den > 0 always (the exp terms are positive), so approx_fast is safe
        nc.vector.reciprocal_approx_fast(rec[:], den[:])
        rb = norm.tile([64, 512], F32, tag=f"rb{i}", name=f"rb{i}")
        nc.gpsimd.partition_broadcast(rb[:], rec[:])
        nc.vector.tensor_mul(dst, o_ps[0:64, :], rb[:])

    def normalize2(o_ps, dst):
        # head 2: denominator already replicated across PSUM rows 64:128.
        # The copy rides scalar (free right after the last exp) so only
        # reciprocal+multiply occupy the vector queue.
        den = norm.tile([64, 512], F32, tag="den2", name="den2")
        nc.scalar.copy(den[:], o_ps[64:128, :])
        rec = norm.tile([64, 512], F32, tag="rec2", name="rec2")
        nc.vector.reciprocal_approx_fast(rec[:], den[:])
        nc.vector.tensor_mul(dst, o_ps[0:64, :], rec[:])

    emit_qkv_m0(0)
    emit_qkv_m12_rest(0, emit_qkv_m1_mm(0))
    emit_qkv_v(0)
    for bi in range(NBI):
        ntj = 4 * bi + 4
        o01 = [ps_o.tile([65, 512], F32, tag=f"o{h}", name=f"o{h}") for h in range(2)]
        o2 = ps_o.tile([128, 512], F32, tag="o2", name="o2")
        OT01 = ot_sb.tile([128, 512], F16, tag="OT01", name="OT01")
        OT2 = ot_sb.tile([64, 512], F16, tag="OT2", name="OT2")

        # ---- heads 0/1: S(tj) then AV(tj-1), one exp per j-tile ----
        pend = None  # (e_tile, tj, lo)

        def flush_av():
            e, tj, lo = pend
            for h in range(2):
                if tj - 4 * bi >= 0:
                    nc.vector.tensor_mul(
                        e[:, h * 512 + lo:h * 512 + lo + 128],
                        e[:, h * 512 + lo:h * 512 + lo + 128], triu_sb[:])
                nc.tensor.matmul(
                    o01[h][:, lo:],
                    v_aug[h][:, tj * VW01:(tj + 1) * VW01],
                    e[:, h * 512 + lo:(h + 1) * 512],
                    start=(tj == 0), stop=(tj == ntj - 1))

        for tj in range(ntj):
            dtile = tj - 4 * bi
            lo = max(dtile, 0) * 128
            js = slice(tj * 128, (tj + 1) * 128)
            qs = slice(bi * 512 + lo, (bi + 1) * 512)
            s_ps = ps.tile([128, 1024], F32, tag="s", name="s")
            nc.tensor.matmul(s_ps[:, lo:512], kTp[0:64, js], qTp[0:64, qs],
                             start=True, stop=True)
            nc.tensor.matmul(s_ps[:, 512 + lo:1024], kTp[64:128, js], qTp[64:128, qs],
                             start=True, stop=True)
            e = work.tile([128, 1024], F16, tag="e", name="e")
            if lo == 0:
                nc.scalar.activation(e[:], s_ps[:],
                                     mybir.ActivationFunctionType.Exp, scale=EXP_SCALE)
            else:
                for h in range(2):
                    nc.scalar.activation(
                        e[:, h * 512 + lo:(h + 1) * 512],
                        s_ps[:, h * 512 + lo:(h + 1) * 512],
                        mybir.ActivationFunctionType.Exp, scale=EXP_SCALE)
            if pend is not None:
                flush_av()
            pend = (e, tj, lo)
        flush_av()

        # ---- head 2: paired j-tiles, AV one pair behind ----
        def flush_av2(ep, pp, losp):
            for idx, tj in enumerate(pp):
                if tj - 4 * bi >= 0:
                    nc.vector.tensor_mul(
                        ep[:, idx * 512 + losp[idx]:idx * 512 + losp[idx] + 128],
                        ep[:, idx * 512 + losp[idx]:idx * 512 + losp[idx] + 128],
                        triu_sb[:])
                nc.tensor.matmul(
                    o2[:, losp[idx]:],
                    v_aug[2][:, tj * VW2:(tj + 1) * VW2],
                    ep[:, idx * 512 + losp[idx]:(idx + 1) * 512],
                    start=(tj == 0), stop=(tj == ntj - 1))

        pend2 = None  # (e, pair, los)
        for tj0 in range(0, ntj, 2):
            pair = (tj0, tj0 + 1)
            s_ps = ps.tile([128, 1024], F32, tag="s", name="s2")
            e = work.tile([128, 1024], F16, tag="e", name="e2")
            los = []
            for idx, tj in enumerate(pair):
                lo = max(tj - 4 * bi, 0) * 128
                los.append(lo)
                hs = slice(64 * idx, 64 * idx + 64)
                js = slice(tj * 128, (tj + 1) * 128)
                qs = slice(bi * 512 + lo, (bi + 1) * 512)
                nc.tensor.matmul(
                    s_ps[:, idx * 512 + lo:(idx + 1) * 512],
                    kT2[hs, js], qT2[hs, qs],
                    start=True, stop=True)
            if los[1] == 0:
                nc.scalar.activation(e[:], s_ps[:],
                                     mybir.ActivationFunctionType.Exp, scale=EXP_SCALE)
            else:
                for idx in range(2):
                    nc.scalar.activation(
                        e[:, idx * 512 + los[idx]:(idx + 1) * 512],
                        s_ps[:, idx * 512 + los[idx]:(idx + 1) * 512],
                        mybir.ActivationFunctionType.Exp, scale=EXP_SCALE)
            if pend2 is not None:
                flush_av2(*pend2)
            pend2 = (e, pair, los)
        # drain: the next block's qkv m0+m1 matmuls are emitted BEFORE the
        # final AV2 flush so the PE chews ~2.6us while the last two exps
        # retire (their casts stay after the flush so the vector-queue triu
        # mask that gates AV2 isn't delayed).
        p1 = None
        if bi + 1 < NBI:
            emit_qkv_m0(bi + 1)
            p1 = emit_qkv_m1_mm(bi + 1)
        flush_av2(*pend2)
        if bi + 1 < NBI:
            emit_qkv_m12_rest(bi + 1, p1)
        # normalize chains ahead of the v-loop casts on the vector queue:
        # proj (gated by the muls) is due sooner than v_aug (next block's
        # diagonal AVs, ~10us away).
        normalize01(o01[0], OT01[0:64], 0)
        normalize01(o01[1], OT01[64:128], 1)
        normalize2(o2, OT2[:, :])
        if bi + 1 < NBI:
            emit_qkv_v(bi + 1)

        # ---- partial proj, straight to DRAM (host sums the partials) ----
        # tt pairs share the two rotating ps buffers; all four OT01 matmuls
        # of a pair run before the OT2 ones, hiding the o2 normalize chain
        # behind real PE work (matters for the exposed final block).
        # Casts split across vector+scalar; output DMA halves leave as soon
        # as their own cast lands (sync / gpsimd queues).
        for tp in range(2):
            pjs = []
            for tt in (2 * tp, 2 * tp + 1):
                ts = slice(tt * 128, (tt + 1) * 128)
                pj = ps.tile([128, 1024], F32, tag="s", name="pj")
                pjs.append(pj)
                # matmul outputs may not cross a PSUM bank: 512 cols in
                # bank 0, the remaining 256 in bank 1 of the same tile.
                for on, osz in ((0, 512), (512, 256)):
                    nc.tensor.matmul(
                        pj[:, on:on + osz], OT01[:, ts], wpA_sb[:, on:on + osz],
                        start=True, stop=False)
            for tt in (2 * tp, 2 * tp + 1):
                ts = slice(tt * 128, (tt + 1) * 128)
                pj = pjs[tt - 2 * tp]
                for on, osz in ((0, 512), (512, 256)):
                    nc.tensor.matmul(
                        pj[:, on:on + osz], OT2[:, ts], wpB_sb[:, on:on + osz],
                        start=False, stop=True)
                y_t = ysb.tile([128, D], F16, tag="y_t", name="y_t")
                nc.vector.tensor_copy(y_t[:, 0:384], pj[:, 0:384])
                nc.scalar.copy(y_t[:, 384:D], pj[:, 384:D])
                yrow = slice(bi * 512 + tt * 128, bi * 512 + (tt + 1) * 128)
                nc.sync.dma_start(y[yrow, 0:384], y_t[:, 0:384])
                last = (bi == NBI - 1 and tt == 3)
                (nc.scalar if last else nc.gpsimd).dma_start(
                    y[yrow, 384:D], y_t[:, 384:D])

    for p in reversed(pools):
        p.__exit__(None, None, None)


_NC_CACHE = {}


def _get_nc():
    if "nc" in _NC_CACHE:
        return _NC_CACHE["nc"]
    nc = bacc.Bacc("TRN2", num_devices=NCORES, debug=False)
    aps = {
        "xT": nc.dram_tensor(
            "xT", [128, NBI * KC * 512], F16, kind="ExternalInput").ap(),
        "wqkT": nc.dram_tensor(
            "wqkT", [128, KC * 2 * QK], F16, kind="ExternalInput").ap(),
        "wvT": nc.dram_tensor(
            "wvT", [128, KC * QK], F16, kind="ExternalInput").ap(),
        "wpT": nc.dram_tensor("wpT", [QK, D], F16, kind="ExternalInput").ap(),
        "triu": nc.dram_tensor("triu", [128, 128], F16, kind="ExternalInput").ap(),
        "y": nc.dram_tensor("y", [T, D], F16, kind="ExternalOutput").ap(),
    }
    with tile.TileContext(nc, num_cores=NCORES) as tc:
        _emit(tc, aps)
    nc.compile()
    _NC_CACHE["nc"] = nc
    return nc


def make_in_maps(x, W_qkv, W_proj):
    triu = np.triu(np.ones((128, 128), dtype=np.float16))
    wpT_full = np.ascontiguousarray(W_proj.T).astype(np.float16)  # [in, out]
    in_maps = []
    for r in range(NCORES):
        b, g = r % 2, r // 2
        rs = slice(QK * g, QK * (g + 1))
        wq = W_qkv[0:D][rs]
        wk = W_qkv[D:2 * D][rs]
        wv = W_qkv[2 * D:3 * D][rs]
        # partition-major folds: row p holds k-chunk blocks back to back, so
        # each tensor arrives in a single contiguous-descriptor dma_start.
        wqkT = np.ascontiguousarray(
            np.concatenate([wq, wk], axis=0).T.astype(np.float16)
            .reshape(KC, 128, 2 * QK).transpose(1, 0, 2).reshape(128, KC * 2 * QK))
        wvT = np.ascontiguousarray(
            wv.T.astype(np.float16)
            .reshape(KC, 128, QK).transpose(1, 0, 2).reshape(128, KC * QK))
        wpT = np.ascontiguousarray(wpT_full[rs, :])
        # x: [128, NBI*KC*512]; col (bi*KC+k)*512+t = x[b][bi*512+t, k*128+p]
        xT = np.ascontiguousarray(
            x[b].astype(np.float16).reshape(NBI, 512, KC, 128)
            .transpose(3, 0, 2, 1).reshape(128, NBI * KC * 512))
        in_maps.append({"xT": xT, "wqkT": wqkT, "wvT": wvT,
                        "wpT": wpT, "triu": triu})
    return in_maps


def assemble(results):
    # unshard: per batch, sum the four head-group partial projections
    y = np.zeros((B, T, D), dtype=np.float32)
    for r in range(NCORES):
        b = r % 2
        y[b] += results[r]["y"].astype(np.float32)
    return y


def kernel(**inputs):
    x = np.asarray(inputs["x"], dtype=np.float32)
    W_qkv = np.asarray(inputs["W_qkv"], dtype=np.float32)
    W_proj = np.asarray(inputs["W_proj"], dtype=np.float32)
    nc = _get_nc()
    in_maps = make_in_maps(x, W_qkv, W_proj)
    res = run_bass_kernel_spmd(nc, in_maps, core_ids=list(range(NCORES)))
    return assemble(res.results)



# revision 56
# speedup vs baseline: 1.0442x; 1.0009x over previous
"""Causal self-attention (B=2, T=2048, D=768, H=12) on 8 TRN2 cores.

Sharding: core r handles batch b=r%2 and head-group g=r//2 (3 heads).
Replica groups are parity-matched ({0,2,4,6} batch 0, {1,3,5,7} batch 1):
observed core-boot skew is odd/even structured, so each 4-core group
syncs only among cores that boot together, and the auto start-barrier
(AllGather over merged replica groups) never couples the two groups.

Per query block bi (512 tokens), fully pipelined:
  1. qkv projection for the block (this core's 3 heads).  x is loaded
     from a host-blocked [4*768, 512] layout so every DMA is contiguous.
  2. attention: S^T = K.Q with keys on partitions, heads 0/1 sharing one
     [128,1024] PSUM strip (one exp ACT covers both), head 2 pairing
     consecutive j-tiles.  The AV matmul for j-tile tj-1 is emitted
     after the S matmuls for tj so the in-order PE never waits on the
     scalar-engine exp.  exp(S/8) without max-subtraction; denominator
     via a ones-row appended to V (O^T = V_aug^T E).
  3. normalize O^T out of PSUM (vector reciprocal on the [1,512]
     denominator row, gpsimd partition-broadcast, vector multiply);
     qkv(bi+1) is emitted before proj(bi) so the PE chews on it while
     the normalize chain runs on the other engines.
  4. partial proj over this core's 192 features -> y_part [512,768] f16,
     DMA'd straight to DRAM.  No collectives at all: the host-side
     assemble() sums the four head-group partials per batch in fp32
     (the unshard step), so cores never synchronize and per-core boot
     skew cannot inflate the max-core exec time.
"""

import numpy as np

import concourse.bass as bass
import concourse.bacc as bacc
import concourse.mybir as mybir
import concourse.tile as tile
from concourse.bass_utils import run_bass_kernel_spmd

F32 = mybir.dt.float32
F16 = mybir.dt.float16

B, T, D = 2, 2048, 768
H, DH = 12, 64
NCORES = 8
HPC = H // 4          # heads per core = 3
QK = HPC * DH         # 192 rows of q (or k) per core
KC = D // 128         # 6 contraction chunks
NBI = T // 512        # 4 query blocks

EXP_SCALE = 1.0 / np.sqrt(DH)  # 0.125
# v_aug column strides.  Heads 0/1: 64 v dims + one ones column (M=65 AV
# matmuls are cheaper; their normalize chain overlaps the head-2 section).
# Head 2: 64 v dims + 64 replicated ones columns, so its AV lands the
# denominator broadcast across PSUM rows 64:128 and the (exposed, block-
# boundary) normalize collapses to copy+reciprocal+multiply with no
# gpsimd partition_broadcast in the chain.
VW01 = 65
VW2 = 128


def _emit(tc, aps):
    nc = tc.nc
    xT, wqkT, wvT, wpT, triu, y = (
        aps["xT"], aps["wqkT"], aps["wvT"], aps["wpT"], aps["triu"], aps["y"])

    pools = []

    def pool(name, bufs, space="SBUF"):
        p = tc.tile_pool(name=name, bufs=bufs, space=space)
        pools.append(p)
        return p.__enter__()

    consts = pool("consts", 1)
    xw = pool("xw", 1)
    qk_sb = pool("qk_sb", 1)
    v_sb = pool("v_sb", 1)
    work = pool("work", 3)
    norm = pool("norm", 2)
    ot_sb = pool("ot_sb", 2)
    ysb = pool("ysb", 4)
    ps = pool("ps", 2, space="PSUM")
    ps_o = pool("ps_o", 1, space="PSUM")
    ps_v = pool("ps_v", 1, space="PSUM")

    # ---- input loads: everything host-swizzled to partition-major so each
    # tensor arrives in ONE dma_start (descriptor streams stay contiguous and
    # the per-call issue cost on the engine queues collapses).
    # x layout: [128, NBI*KC*512]; col (bi*KC+k)*512+t = x[b][bi*512+t, k*128+p]
    x_sb = xw.tile([128, NBI * KC * 512], F16, tag="x", name="x")
    wqk_sb = consts.tile([128, KC * 2 * QK], F16, tag="wqk", name="wqk")
    wv_sb = consts.tile([128, KC * QK], F16, tag="wv", name="wv")

    def xcol(bi, k):  # column base of (block, k-chunk) in x_sb
        return (bi * KC + k) * 512

    # critical path first: wqk k-chunk 0 + per-chunk x(b0) calls alternating
    # sync/scalar, so chunk k lands k-th and the k-outer first-block qkv
    # consumes each on arrival; x blocks 1-3 are issued from the BACK of the
    # gpsimd queue (after the boot memsets) so their 2.4 MB never competes
    # with block-0 latency.
    nc.gpsimd.dma_start(wqk_sb[:, 0:2 * QK], wqkT[:, 0:2 * QK])
    nc.sync.dma_start(x_sb[:, 0:512], xT[:, 0:512])
    nc.scalar.dma_start(x_sb[:, 512:1024], xT[:, 512:1024])
    nc.gpsimd.dma_start(wqk_sb[:, 2 * QK:], wqkT[:, 2 * QK:])
    for k in range(2, KC):
        (nc.sync if k % 2 == 0 else nc.scalar).dma_start(
            x_sb[:, k * 512:(k + 1) * 512], xT[:, k * 512:(k + 1) * 512])
    triu_sb = consts.tile([128, 128], F16, tag="triu", name="triu")
    nc.gpsimd.dma_start(wv_sb[:], wvT[:, :])
    nc.gpsimd.dma_start(triu_sb[:], triu[:, :])
    wpA_sb = consts.tile([128, D], F16, tag="wpA", name="wpA")
    wpB_sb = consts.tile([64, D], F16, tag="wpB", name="wpB")

    # ---- persistent q/k/v tiles ----
    # heads 0/1 packed into [128, T] (rows 0-63 / 64-127); head 2 in [64, T].
    qTp = qk_sb.tile([128, T], F16, tag="qTp", name="qTp")
    kTp = qk_sb.tile([128, T], F16, tag="kTp", name="kTp")
    # head 2 q/k live in BOTH partition halves: paired j-tiles then hit
    # disjoint PE row groups (h0/h64) and run concurrently, like heads 0/1.
    qT2 = qk_sb.tile([128, T], F16, tag="qT2", name="qT2")
    kT2 = qk_sb.tile([128, T], F16, tag="kT2", name="kT2")
    VWS = [VW01, VW01, VW2]
    v_aug = [v_sb.tile([128, (T // 128) * VWS[h]], F16, tag=f"v{h}", name=f"v{h}")
             for h in range(HPC)]
    # static ones columns, set once at boot; only the v-data halves are
    # rewritten per block.  Emitted on the gpsimd queue ahead of the
    # deferred x DMAs below.
    for h in range(HPC):
        vw = VWS[h]
        for tt in range(T // 128):
            nc.gpsimd.memset(v_aug[h][:, tt * vw + 64:(tt + 1) * vw], 1.0)
    # deferred bulk loads ride the back of the gpsimd queue: x blocks 1-3
    # with the proj weights (first needed ~25us in) slotted between
    nc.gpsimd.dma_start(
        x_sb[:, xcol(1, 0):xcol(2, 0)], xT[:, xcol(1, 0):xcol(2, 0)])
    nc.gpsimd.dma_start(wpA_sb[:], wpT[0:128, :])
    nc.gpsimd.dma_start(wpB_sb[:], wpT[128:QK, :])
    for bi in range(2, NBI):
        nc.gpsimd.dma_start(
            x_sb[:, xcol(bi, 0):xcol(bi + 1, 0)], xT[:, xcol(bi, 0):xcol(bi + 1, 0)])

    def _qk_mm(bi, m):
        p = ps.tile([128, 1024], F32, tag="s", name="qkps")[:, 0:512]
        for k in range(KC):
            nc.tensor.matmul(
                p[:],
                wqk_sb[:, k * 2 * QK + m * 128:k * 2 * QK + (m + 1) * 128],
                x_sb[:, xcol(bi, k):xcol(bi, k) + 512],
                start=(k == 0), stop=(k == KC - 1))
        return p

    def emit_qkv_m0(bi):
        # m0's cast rides scalar, so emitting it before the last AV2 flush
        # leaves the vector queue free for the triu mask that gates AV2.
        ns = slice(bi * 512, (bi + 1) * 512)
        p = _qk_mm(bi, 0)
        nc.scalar.copy(qTp[:, ns], p[:])

    def emit_qkv_m1_mm(bi):
        # m1 accumulates in the ps_v bank: unlike the rotating ps pool, its
        # WAR is on the long-done v casts, so these matmuls can run during
        # the S2 exp drain when emitted before the final AV2 flush.
        p = ps_v.tile([128, 512], F32, tag="v", name="m1ps")
        for k in range(KC):
            nc.tensor.matmul(
                p[:],
                wqk_sb[:, k * 2 * QK + 128:k * 2 * QK + 256],
                x_sb[:, xcol(bi, k):xcol(bi, k) + 512],
                start=(k == 0), stop=(k == KC - 1))
        return p

    def emit_qkv_m12_rest(bi, p1):
        ns = slice(bi * 512, (bi + 1) * 512)
        nc.vector.tensor_copy(qT2[0:64, ns], p1[0:64, :])
        nc.vector.tensor_copy(qT2[64:128, ns], p1[0:64, :])
        nc.vector.tensor_copy(kTp[0:64, ns], p1[64:128, :])
        p = _qk_mm(bi, 2)
        nc.scalar.copy(kTp[64:128, ns], p[0:64, :])
        nc.scalar.copy(kT2[0:64, ns], p[64:128, :])
        nc.scalar.copy(kT2[64:128, ns], p[64:128, :])

    def emit_qkv_v(bi):
        for tt in range(bi * 4, bi * 4 + 4):
            # own single-bank pool, two region slots: never contends with the
            # m-strip casts for PSUM write-after-read
            p = ps_v.tile([128, 512], F32, tag="v", name="vps")[
                :, (tt % 2) * 256:(tt % 2) * 256 + QK]
            u = tt % 4
            for k in range(KC):
                nc.tensor.matmul(
                    p[:],
                    x_sb[:, xcol(bi, k) + u * 128:xcol(bi, k) + (u + 1) * 128],
                    wv_sb[:, k * QK:(k + 1) * QK],
                    start=(k == 0), stop=(k == KC - 1))
            for h in range(HPC):
                nc.vector.tensor_copy(
                    v_aug[h][:, tt * VWS[h]:tt * VWS[h] + 64],
                    p[:, h * 64:(h + 1) * 64])

    def normalize01(o_ps, dst, i):
        # heads 0/1: single denominator row.  Stays entirely off the scalar
        # queue (exp must not sit behind these) and overlaps the head-2
        # section / next-block qkv on the PE.
        den = norm.tile([1, 512], F32, tag=f"den{i}", name=f"den{i}")
        nc.vector.tensor_copy(den[:], o_ps[64:65, :])
        rec = norm.tile([1, 512], F32, tag=f"rec{i}", name=f"rec{i}")
        # den > 0 always (the exp terms are positive), so approx_fast is safe
        nc.vector.reciprocal_approx_fast(rec[:], den[:])
        rb = norm.tile([64, 512], F32, tag=f"rb{i}", name=f"rb{i}")
        nc.gpsimd.partition_broadcast(rb[:], rec[:])
        nc.vector.tensor_mul(dst, o_ps[0:64, :], rb[:])

    def normalize2(o_ps, dst):
        # head 2: denominator already replicated across PSUM rows 64:128.
        # The copy rides scalar (free right after the last exp) so only
        # reciprocal+multiply occupy the vector queue.
        den = norm.tile([64, 512], F32, tag="den2", name="den2")
        nc.scalar.copy(den[:], o_ps[64:128, :])
        rec = norm.tile([64, 512], F32, tag="rec2", name="rec2")
        nc.vector.reciprocal_approx_fast(rec[:], den[:])
        nc.vector.tensor_mul(dst, o_ps[0:64, :], rec[:])

    # block 0: interleave the three m-matmuls per k-chunk so each arriving
    # x chunk (the startup DMA trickle) immediately feeds ~650ns of PE work
    ns0 = slice(0, 512)
    p_m0 = ps.tile([128, 1024], F32, tag="s", name="qk0A")[:, 0:512]
    p_m1 = ps_v.tile([128, 512], F32, tag="v", name="qk0m1")
    p_m2 = ps.tile([128, 1024], F32, tag="s", name="qk0B")[:, 0:512]
    for k in range(KC):
        for m, p in ((0, p_m0), (1, p_m1), (2, p_m2)):
            nc.tensor.matmul(
                p[:],
                wqk_sb[:, k * 2 * QK + m * 128:k * 2 * QK + (m + 1) * 128],
                x_sb[:, k * 512:k * 512 + 512],
                start=(k == 0), stop=(k == KC - 1))
    nc.scalar.copy(qTp[:, ns0], p_m0[:])
    nc.vector.tensor_copy(qT2[0:64, ns0], p_m1[0:64, :])
    nc.vector.tensor_copy(qT2[64:128, ns0], p_m1[0:64, :])
    nc.vector.tensor_copy(kTp[0:64, ns0], p_m1[64:128, :])
    nc.scalar.copy(kTp[64:128, ns0], p_m2[0:64, :])
    nc.scalar.copy(kT2[0:64, ns0], p_m2[64:128, :])
    nc.scalar.copy(kT2[64:128, ns0], p_m2[64:128, :])
    emit_qkv_v(0)
    for bi in range(NBI):
        ntj = 4 * bi + 4
        o01 = [ps_o.tile([65, 512], F32, tag=f"o{h}", name=f"o{h}") for h in range(2)]
        o2 = ps_o.tile([128, 512], F32, tag="o2", name="o2")
        OT01 = ot_sb.tile([128, 512], F16, tag="OT01", name="OT01")
        OT2 = ot_sb.tile([64, 512], F16, tag="OT2", name="OT2")

        # ---- heads 0/1: S(tj) then AV(tj-1), one exp per j-tile ----
        pend = None  # (e_tile, tj, lo)

        def flush_av():
            e, tj, lo = pend
            for h in range(2):
                if tj - 4 * bi >= 0:
                    nc.vector.tensor_mul(
                        e[:, h * 512 + lo:h * 512 + lo + 128],
                        e[:, h * 512 + lo:h * 512 + lo + 128], triu_sb[:])
                nc.tensor.matmul(
                    o01[h][:, lo:],
                    v_aug[h][:, tj * VW01:(tj + 1) * VW01],
                    e[:, h * 512 + lo:(h + 1) * 512],
                    start=(tj == 0), stop=(tj == ntj - 1))

        for tj in range(ntj):
            dtile = tj - 4 * bi
            lo = max(dtile, 0) * 128
            js = slice(tj * 128, (tj + 1) * 128)
            qs = slice(bi * 512 + lo, (bi + 1) * 512)
            s_ps = ps.tile([128, 1024], F32, tag="s", name="s")
            nc.tensor.matmul(s_ps[:, lo:512], kTp[0:64, js], qTp[0:64, qs],
                             start=True, stop=True)
            nc.tensor.matmul(s_ps[:, 512 + lo:1024], kTp[64:128, js], qTp[64:128, qs],
                             start=True, stop=True)
            e = work.tile([128, 1024], F16, tag="e", name="e")
            if lo == 0:
                nc.scalar.activation(e[:], s_ps[:],
                                     mybir.ActivationFunctionType.Exp, scale=EXP_SCALE)
            else:
                for h in range(2):
                    nc.scalar.activation(
                        e[:, h * 512 + lo:(h + 1) * 512],
                        s_ps[:, h * 512 + lo:(h + 1) * 512],
                        mybir.ActivationFunctionType.Exp, scale=EXP_SCALE)
            if pend is not None:
                flush_av()
            pend = (e, tj, lo)
        flush_av()
        # o01 chains start now (vector copies; scalar exps untouched) so
        # their muls are long done when proj needs OT01 — even in the last
        # block, where the S2 section is all that remains to overlap.
        normalize01(o01[0], OT01[0:64], 0)
        normalize01(o01[1], OT01[64:128], 1)

        # ---- head 2: paired j-tiles, AV one pair behind ----
        def flush_av2(ep, pp, losp):
            for idx, tj in enumerate(pp):
                if tj - 4 * bi >= 0:
                    nc.vector.tensor_mul(
                        ep[:, idx * 512 + losp[idx]:idx * 512 + losp[idx] + 128],
                        ep[:, idx * 512 + losp[idx]:idx * 512 + losp[idx] + 128],
                        triu_sb[:])
                nc.tensor.matmul(
                    o2[:, losp[idx]:],
                    v_aug[2][:, tj * VW2:(tj + 1) * VW2],
                    ep[:, idx * 512 + losp[idx]:(idx + 1) * 512],
                    start=(tj == 0), stop=(tj == ntj - 1))

        pend2 = None  # (e, pair, los)
        for tj0 in range(0, ntj, 2):
            pair = (tj0, tj0 + 1)
            s_ps = ps.tile([128, 1024], F32, tag="s", name="s2")
            e = work.tile([128, 1024], F16, tag="e", name="e2")
            los = []
            for idx, tj in enumerate(pair):
                lo = max(tj - 4 * bi, 0) * 128
                los.append(lo)
                hs = slice(64 * idx, 64 * idx + 64)
                js = slice(tj * 128, (tj + 1) * 128)
                qs = slice(bi * 512 + lo, (bi + 1) * 512)
                nc.tensor.matmul(
                    s_ps[:, idx * 512 + lo:(idx + 1) * 512],
                    kT2[hs, js], qT2[hs, qs],
                    start=True, stop=True)
            if los[1] == 0:
                nc.scalar.activation(e[:], s_ps[:],
                                     mybir.ActivationFunctionType.Exp, scale=EXP_SCALE)
            else:
                for idx in range(2):
                    nc.scalar.activation(
                        e[:, idx * 512 + los[idx]:(idx + 1) * 512],
                        s_ps[:, idx * 512 + los[idx]:(idx + 1) * 512],
                        mybir.ActivationFunctionType.Exp, scale=EXP_SCALE)
            if pend2 is not None:
                flush_av2(*pend2)
            pend2 = (e, pair, los)
        # drain: the next block's qkv m0+m1 matmuls are emitted BEFORE the
        # final AV2 flush so the PE chews ~2.6us while the last two exps
        # retire (their casts stay after the flush so the vector-queue triu
        # mask that gates AV2 isn't delayed).
        p1 = None
        if bi + 1 < NBI:
            emit_qkv_m0(bi + 1)
            p1 = emit_qkv_m1_mm(bi + 1)
        flush_av2(*pend2)
        # normalize2 ahead of the m1 casts on the vector queue: its mul
        # gates the exposed proj OT2 matmuls.
        normalize2(o2, OT2[:, :])
        if bi + 1 < NBI:
            emit_qkv_m12_rest(bi + 1, p1)
            emit_qkv_v(bi + 1)

        # ---- partial proj, straight to DRAM (host sums the partials) ----
        # tt pairs share the two rotating ps buffers; all four OT01 matmuls
        # of a pair run before the OT2 ones, hiding the o2 normalize chain
        # behind real PE work (matters for the exposed final block).
        # Casts split across vector+scalar; output DMA halves leave as soon
        # as their own cast lands (sync / gpsimd queues).
        for tp in range(2):
            pjs = []
            for tt in (2 * tp, 2 * tp + 1):
                ts = slice(tt * 128, (tt + 1) * 128)
                pj = ps.tile([128, 1024], F32, tag="s", name="pj")
                pjs.append(pj)
                # matmul outputs may not cross a PSUM bank: 512 cols in
                # bank 0, the remaining 256 in bank 1 of the same tile.
                for on, osz in ((0, 512), (512, 256)):
                    nc.tensor.matmul(
                        pj[:, on:on + osz], OT01[:, ts], wpA_sb[:, on:on + osz],
                        start=True, stop=False)
            for tt in (2 * tp, 2 * tp + 1):
                ts = slice(tt * 128, (tt + 1) * 128)
                pj = pjs[tt - 2 * tp]
                for on, osz in ((0, 512), (512, 256)):
                    nc.tensor.matmul(
                        pj[:, on:on + osz], OT2[:, ts], wpB_sb[:, on:on + osz],
                        start=False, stop=True)
                y_t = ysb.tile([128, D], F16, tag="y_t", name="y_t")
                nc.vector.tensor_copy(y_t[:, 0:384], pj[:, 0:384])
                nc.scalar.copy(y_t[:, 384:D], pj[:, 384:D])
                yrow = slice(bi * 512 + tt * 128, bi * 512 + (tt + 1) * 128)
                nc.sync.dma_start(y[yrow, 0:384], y_t[:, 0:384])
                last = (bi == NBI - 1 and tt == 3)
                (nc.scalar if last else nc.gpsimd).dma_start(
                    y[yrow, 384:D], y_t[:, 384:D])

    for p in reversed(pools):
        p.__exit__(None, None, None)


_NC_CACHE = {}


def _get_nc():
    if "nc" in _NC_CACHE:
        return _NC_CACHE["nc"]
    nc = bacc.Bacc("TRN2", num_devices=NCORES, debug=False)
    aps = {
        "xT": nc.dram_tensor(
            "xT", [128, NBI * KC * 512], F16, kind="ExternalInput").ap(),
        "wqkT": nc.dram_tensor(
            "wqkT", [128, KC * 2 * QK], F16, kind="ExternalInput").ap(),
        "wvT": nc.dram_tensor(
            "wvT", [128, KC * QK], F16, kind="ExternalInput").ap(),
        "wpT": nc.dram_tensor("wpT", [QK, D], F16, kind="ExternalInput").ap(),
        "triu": nc.dram_tensor("triu", [128, 128], F16, kind="ExternalInput").ap(),
        "y": nc.dram_tensor("y", [T, D], F16, kind="ExternalOutput").ap(),
    }
    with tile.TileContext(nc, num_cores=NCORES) as tc:
        _emit(tc, aps)
    nc.compile()
    _NC_CACHE["nc"] = nc
    return nc


def make_in_maps(x, W_qkv, W_proj):
    triu = np.triu(np.ones((128, 128), dtype=np.float16))
    wpT_full = np.ascontiguousarray(W_proj.T).astype(np.float16)  # [in, out]
    in_maps = []
    for r in range(NCORES):
        b, g = r % 2, r // 2
        rs = slice(QK * g, QK * (g + 1))
        wq = W_qkv[0:D][rs]
        wk = W_qkv[D:2 * D][rs]
        wv = W_qkv[2 * D:3 * D][rs]
        # partition-major folds: row p holds k-chunk blocks back to back, so
        # each tensor arrives in a single contiguous-descriptor dma_start.
        wqkT = np.ascontiguousarray(
            np.concatenate([wq, wk], axis=0).T.astype(np.float16)
            .reshape(KC, 128, 2 * QK).transpose(1, 0, 2).reshape(128, KC * 2 * QK))
        wvT = np.ascontiguousarray(
            wv.T.astype(np.float16)
            .reshape(KC, 128, QK).transpose(1, 0, 2).reshape(128, KC * QK))
        wpT = np.ascontiguousarray(wpT_full[rs, :])
        # x: [128, NBI*KC*512]; col (bi*KC+k)*512+t = x[b][bi*512+t, k*128+p]
        xT = np.ascontiguousarray(
            x[b].astype(np.float16).reshape(NBI, 512, KC, 128)
            .transpose(3, 0, 2, 1).reshape(128, NBI * KC * 512))
        in_maps.append({"xT": xT, "wqkT": wqkT, "wvT": wvT,
                        "wpT": wpT, "triu": triu})
    return in_maps


def assemble(results):
    # unshard: per batch, sum the four head-group partial projections
    y = np.zeros((B, T, D), dtype=np.float32)
    for r in range(NCORES):
        b = r % 2
        y[b] += results[r]["y"].astype(np.float32)
    return y


def kernel(**inputs):
    x = np.asarray(inputs["x"], dtype=np.float32)
    W_qkv = np.asarray(inputs["W_qkv"], dtype=np.float32)
    W_proj = np.asarray(inputs["W_proj"], dtype=np.float32)
    nc = _get_nc()
    in_maps = make_in_maps(x, W_qkv, W_proj)
    res = run_bass_kernel_spmd(nc, in_maps, core_ids=list(range(NCORES)))
    return assemble(res.results)

